# revision 1
# baseline (speedup 1.0000x reference)
"""Trainium2 Bass kernel for single-head cross-attention (v2: V exchanged).

Reference computation (B=4, Sq=Skv=2048, D=1024, fp32):
    Q = query @ Wq + bq ; K = key @ Wk + bk ; V = value @ Wv + bv
    out = softmax(Q K^T / sqrt(D)) V @ Wo + bo

Sharding: 8 shards = (batch b in 0..3) x (query half h in 0..1); core
c = 2*b + h computes output rows [h*1024,(h+1)*1024) of batch b. Each
core projects only its OWN kv-half of both K and V and the pair
exchanges halves with two AllGathers (K first, then V).

Attention runs in two kv passes so the collectives hide under compute:
pass 1 uses the locally-projected halves straight from SBUF (no
collective dependency); pass 2 uses the other core's halves. Since the
softmax here is unnormalized (divide by the sum at the very end), the
two passes just accumulate - no flash-style rescaling.

SPMD cannot address "the other rank's slot" of an AllGather result, so
each core gets a per-core one-hot mask input m with m[j] = (j != rank)
and computes other = gathered[0]*m0 + gathered[1]*m1 on the (idle)
Vector engine.

Dataflow is transpose-free on device: the host ships activations
feature-major (qT/kT/vT = x.T, contiguous) so every matmul's contraction
dim lands on SBUF partitions:
    Q^T[e,q]   = Wq.T @ qT         (lhsT=Wq,   rhs=qT)
    K^T[e,kv]  = Wk.T @ kT         (lhsT=Wk,   rhs=kT)   own half -> AllGather
    V[kv,dv]   = vT.T @ Wv         (lhsT=vT,   rhs=Wv)   own half -> AllGather
    S^T[kv,q]  = K @ Q^T           (lhsT=K^T,  rhs=Q^T)
    A^T        = exp(S^T/32)                    (unnormalized; scores are O(1))
    O^T[dv,q]  = V.T @ A^T         (lhsT=V,    rhs=A^T)
    sums[q,1]  = A @ ones          (lhsT=A^T,  rhs=ones)
    F[q,f]     = O @ Wo            (lhsT=O^T,  rhs=Wo)
    out        = F * (1/sums) + (bv @ Wo + bo)  (softmax denom commutes
                                                 through V and Wo; bv is
                                                 folded into the output
                                                 bias on the host)
"""

import sys

if "/opt/trn_rl_repo" not in sys.path:
    sys.path.insert(0, "/opt/trn_rl_repo")

from contextlib import ExitStack

import ml_dtypes
import numpy as np

import concourse.bass as bass
import concourse.mybir as mybir
import concourse.tile as tile
from concourse import bacc
from concourse.bass_utils import run_bass_kernel_spmd

B, SQ, SKV, D = 4, 2048, 2048, 1024
NCORES = 8
QL = SQ // 2  # local query rows per core
KVH = SKV // 2  # own kv half per core
P = 128
DC = D // P  # feature chunks (8)
KVC = SKV // P  # kv chunks (16)
KVHC = KVH // P  # kv chunks per half (8)
N5 = 512
F32 = mybir.dt.float32
CDT = mybir.dt.bfloat16  # on-device compute dtype for matmul operands
NP_CDT = ml_dtypes.bfloat16
SCALE = 1.0 / 32.0  # 1/sqrt(D)

AF = mybir.ActivationFunctionType
ALU = mybir.AluOpType
GROUPS = [[0, 1], [2, 3], [4, 5], [6, 7]]


def _build_tile(ctx: ExitStack, tc, aps, dram):
    nc = tc.nc
    qT, kT, vT, wq, wk, wv, wo, bq, bk, bo2, mask, out = aps
    kg_half, kg_full, vg_half, vg_full = dram

    wpool_cm = tc.tile_pool(name="wpool", bufs=1)  # wk/wv/wq: closed pre-attention
    spool_cm = tc.tile_pool(name="streams", bufs=3)  # input streams: closed too
    big = ctx.enter_context(tc.tile_pool(name="big", bufs=1))
    xchg = ctx.enter_context(tc.tile_pool(name="xchg", bufs=1))
    evac = ctx.enter_context(tc.tile_pool(name="evac", bufs=4))
    psum = ctx.enter_context(tc.tile_pool(name="psum", bufs=4, space="PSUM"))
    psum_s = ctx.enter_context(tc.tile_pool(name="psum_s", bufs=2, space="PSUM"))
    wpool = wpool_cm.__enter__()
    streams = spool_cm.__enter__()

    # Weights arrive one 128-row d-chunk per DMA, in the order compute
    # consumes them, so the PE isn't stalled behind bulk weight traffic.
    def w_chunks(ap, tag, pool):
        return [
            (
                pool.tile([P, D], CDT, tag=f"{tag}{dc}", name=f"{tag}{dc}"),
                ap[dc * P : (dc + 1) * P, :],
            )
            for dc in range(DC)
        ]

    def load_chunks(tiles):
        for t, src in tiles:
            nc.sync.dma_start(out=t, in_=src)

    def load_b(ap, tag, pool):
        t = pool.tile([P, DC], F32, tag=tag, name=tag)
        nc.sync.dma_start(out=t, in_=ap.rearrange("(c p) -> p c", p=P))
        return t

    kT_r = kT.rearrange("(c p) n -> p c n", p=P)
    qT_r = qT.rearrange("(c p) n -> p c n", p=P)
    vT_r = vT.rearrange("(c p) n -> p c n", p=P)

    mask_s = big.tile([P, 2], F32, tag="mask", name="mask")
    nc.sync.dma_start(out=mask_s, in_=mask)

    wk_c = w_chunks(wk, "wk", wpool)
    # First input tile split per d-chunk: the first matmul only waits on
    # wk chunk 0 + one 128x512 slice instead of 3 MiB of queued DMA.
    k_in0 = streams.tile([P, DC, N5], CDT, tag="xin")
    for dc in range(DC):
        nc.sync.dma_start(out=wk_c[dc][0], in_=wk_c[dc][1])
        nc.sync.dma_start(out=k_in0[:, dc, :], in_=kT_r[:, dc, 0:N5])
    bk_s = load_b(bk, "bk", wpool)
    k_in1 = streams.tile([P, DC, N5], CDT, tag="xin")
    nc.sync.dma_start(out=k_in1, in_=kT_r[:, :, N5 : 2 * N5])

    # ---- K^T own half -> kOwn (kept in SBUF for pass-1 attention) ----------
    kOwn = big.tile([P, DC, KVH], CDT, tag="kOwn")  # K^T own: [e%128, e//128, kv]

    def k_block(x_in, j):
        for ec in range(DC):
            ps = psum.tile([P, N5], F32, tag="mm")
            for dc in range(DC):
                nc.tensor.matmul(
                    ps,
                    lhsT=wk_c[dc][0][:, ec * P : (ec + 1) * P],
                    rhs=x_in[:, dc, :],
                    start=(dc == 0),
                    stop=(dc == DC - 1),
                )
            nc.scalar.activation(
                out=kOwn[:, ec, j * N5 : (j + 1) * N5],
                in_=ps,
                func=AF.Identity,
                bias=bk_s[:, ec : ec + 1],
                scale=1.0,
            )

    k_block(k_in0, 0)
    k_block(k_in1, 1)

    # Dump own half and AllGather the pair's halves. Dumps/reloads ride the
    # ACT HWDGE ring (nc.scalar) so they don't queue behind the input
    # streams on the SP ring.
    for j in range(KVH // N5):
        nc.scalar.dma_start(
            out=kg_half[:, :, j * N5 : (j + 1) * N5],
            in_=kOwn[:, :, j * N5 : (j + 1) * N5],
        )
    nc.gpsimd.collective_compute(
        "AllGather",
        ALU.bypass,
        replica_groups=GROUPS,
        ins=[kg_half[:]],
        outs=[kg_full[:]],
    )
    # K reloads ride the gpsimd ring, emitted BEFORE the V collective so they
    # wait only on the K one. The gpsimd queue blocking until ~K-done costs
    # nothing: the CC serializes the V collective behind K anyway.
    kOther = big.tile([P, DC, KVH], CDT, tag="kOther")
    kre = []
    for t in range(2):
        sl = slice(t * (KVH // 2), (t + 1) * (KVH // 2))
        g0 = xchg.tile([P, DC, KVH // 2], CDT, tag="kg0")
        g1 = xchg.tile([P, DC, KVH // 2], CDT, tag="kg1")
        nc.gpsimd.dma_start(out=g0, in_=kg_full[0, :, :, sl])
        nc.gpsimd.dma_start(out=g1, in_=kg_full[1, :, :, sl])
        kre.append((sl, g0, g1))

    # ---- V projection (own kv half) -----------------------------------------
    wv_c = w_chunks(wv, "wv", wpool)
    load_chunks(wv_c)
    vOwn = big.tile([P, KVHC, D], CDT, tag="vOwn")  # V own: [kv%128, kv//128, dv]
    for j in range(KVH // N5):
        v_in = streams.tile([P, DC, N5], CDT, tag="xin")
        nc.sync.dma_start(out=v_in, in_=vT_r[:, :, j * N5 : (j + 1) * N5])
        for sub in range(N5 // P):
            c = j * (N5 // P) + sub
            for nv in range(D // N5):
                ps = psum.tile([P, N5], F32, tag="mm")
                for dc in range(DC):
                    nc.tensor.matmul(
                        ps,
                        lhsT=v_in[:, dc, sub * P : (sub + 1) * P],
                        rhs=wv_c[dc][0][:, nv * N5 : (nv + 1) * N5],
                        start=(dc == 0),
                        stop=(dc == DC - 1),
                    )
                nc.vector.tensor_copy(
                    out=vOwn[:, c, nv * N5 : (nv + 1) * N5], in_=ps
                )
    for h in range(2):
        nc.scalar.dma_start(
            out=vg_half[:, h * (KVHC // 2) : (h + 1) * (KVHC // 2), :],
            in_=vOwn[:, h * (KVHC // 2) : (h + 1) * (KVHC // 2), :],
        )
    nc.gpsimd.collective_compute(
        "AllGather",
        ALU.bypass,
        replica_groups=GROUPS,
        ins=[vg_half[:]],
        outs=[vg_full[:]],
    )

    ones = big.tile([P, 1], CDT, tag="ones")
    nc.vector.memset(ones, 1.0)
    vOther = big.tile([P, KVHC, D], CDT, tag="vOther")
    vre = []
    for t in range(2):
        sl = slice(t * (KVHC // 2), (t + 1) * (KVHC // 2))
        g0 = xchg.tile([P, KVHC // 2, D], CDT, tag="vg0")
        g1 = xchg.tile([P, KVHC // 2, D], CDT, tag="vg1")
        nc.gpsimd.dma_start(out=g0, in_=vg_full[0, :, sl, :])
        nc.gpsimd.dma_start(out=g1, in_=vg_full[1, :, sl, :])
        vre.append((sl, g0, g1))

    # ---- Q^T projection (overlaps the collectives) ---------------------------
    wq_c = w_chunks(wq, "wq", wpool)
    load_chunks(wq_c)
    bq_s = load_b(bq, "bq", wpool)
    qTo = big.tile([P, DC, QL], CDT, tag="qTo")  # Q^T: [e%128, e//128, q]
    for j in range(QL // N5):
        x_in = streams.tile([P, DC, N5], CDT, tag="xin")
        nc.sync.dma_start(out=x_in, in_=qT_r[:, :, j * N5 : (j + 1) * N5])
        for ec in range(DC):
            ps = psum.tile([P, N5], F32, tag="mm")
            for dc in range(DC):
                nc.tensor.matmul(
                    ps,
                    lhsT=wq_c[dc][0][:, ec * P : (ec + 1) * P],
                    rhs=x_in[:, dc, :],
                    start=(dc == 0),
                    stop=(dc == DC - 1),
                )
            nc.scalar.activation(
                out=qTo[:, ec, j * N5 : (j + 1) * N5],
                in_=ps,
                func=AF.Identity,
                bias=bq_s[:, ec : ec + 1],
                scale=1.0,
            )

    spool_cm.__exit__(None, None, None)
    wpool_cm.__exit__(None, None, None)
    # weights pool opens only now, reusing the space wpool/streams vacated.
    weights = ctx.enter_context(tc.tile_pool(name="weights", bufs=1))
    wo_c = w_chunks(wo, "wo", weights)
    load_chunks(wo_c)
    bo2_s = weights.tile([P, D], F32, tag="bo2")
    bo2_bcast = bass.AP(tensor=bo2.tensor, offset=bo2.offset, ap=[[0, P], bo2.ap[0]])
    nc.sync.dma_start(out=bo2_s, in_=bo2_bcast)

    # other = g0*m0 + g1*m1 on Vector (idle while Scalar does the exps).
    def k_combine():
        for sl, g0, g1 in kre:
            nc.vector.tensor_scalar_mul(kOther[:, :, sl], g0, mask_s[:, 0:1])
            nc.vector.scalar_tensor_tensor(
                out=kOther[:, :, sl],
                in0=g1,
                scalar=mask_s[:, 1:2],
                in1=kOther[:, :, sl],
                op0=ALU.mult,
                op1=ALU.add,
            )

    def v_combine():
        for sl, g0, g1 in vre:
            nc.vector.tensor_scalar_mul(vOther[:, sl, :], g0, mask_s[:, 0:1])
            nc.vector.scalar_tensor_tensor(
                out=vOther[:, sl, :],
                in0=g1,
                scalar=mask_s[:, 1:2],
                in1=vOther[:, sl, :],
                op0=ALU.mult,
                op1=ALU.add,
            )

    # ---- attention: two kv passes per 512-query block ------------------------
    attn_pool = ctx.enter_context(tc.tile_pool(name="attn", bufs=2))
    NQB = QL // N5

    def scores_exp(kv_src, qb):
        """S^T = kv_src @ Q^T chunk, exp -> attnT; sums accumulate per qb."""
        attnT = attn_pool.tile([P, KVHC, N5], CDT, tag="attnT")
        for c in range(KVHC):
            ps = psum.tile([P, N5], F32, tag="mm")
            for ec in range(DC):
                nc.tensor.matmul(
                    ps,
                    lhsT=kv_src[:, ec, c * P : (c + 1) * P],
                    rhs=qTo[:, ec, qb * N5 : (qb + 1) * N5],
                    start=(ec == 0),
                    stop=(ec == DC - 1),
                )
            nc.scalar.activation(out=attnT[:, c, :], in_=ps, func=AF.Exp, scale=SCALE)
        return attnT

    def sums_acc(attnT, ps_sum):
        for s in range(N5 // P):
            for c in range(KVHC):
                nc.tensor.matmul(
                    ps_sum[:, s : s + 1],
                    lhsT=attnT[:, c, s * P : (s + 1) * P],
                    rhs=ones[:, :1],
                    start=(c == 0),
                    stop=(c == KVHC - 1),
                )

    # ---- pass 1: own kv half straight from SBUF ------------------------------
    sums1 = []
    attnT1 = []
    oacc = []
    for qb in range(NQB):
        aT = scores_exp(kOwn, qb)
        ps_sum = psum_s.tile([P, N5 // P], F32, tag="sums")
        sums_acc(aT, ps_sum)
        s1 = evac.tile([P, N5 // P], F32, tag="s1")
        nc.vector.tensor_copy(out=s1, in_=ps_sum)
        sums1.append(s1)
        attnT1.append(aT)
    # K combine sits here on the Vector queue: Vector is idle during pass-1
    # scores (exp is on Scalar), and kOther is ready well before pass 2.
    k_combine()
    for qb in range(NQB):
        oa = attn_pool.tile([P, DC, N5], CDT, tag="oacc")
        for m in range(DC):
            ps = psum.tile([P, N5], F32, tag="mm")
            for c in range(KVHC):
                nc.tensor.matmul(
                    ps,
                    lhsT=vOwn[:, c, m * P : (m + 1) * P],
                    rhs=attnT1[qb][:, c, :],
                    start=(c == 0),
                    stop=(c == KVHC - 1),
                )
            nc.scalar.activation(
                out=oa[:, m, :], in_=ps, func=AF.Identity, scale=1.0
            )
        oacc.append(oa)
    v_combine()

    # ---- pass 2: other kv half + merge + output projection -------------------
    attnT2 = []
    r_ss = []
    for qb in range(NQB):
        aT = scores_exp(kOther, qb)
        ps_sum = psum_s.tile([P, N5 // P], F32, tag="sums")
        sums_acc(aT, ps_sum)
        tot = evac.tile([P, N5 // P], F32, tag="stot")
        nc.vector.scalar_tensor_tensor(
            out=tot, in0=ps_sum, scalar=1.0, in1=sums1[qb],
            op0=ALU.mult, op1=ALU.add,
        )
        r_s = evac.tile([P, N5 // P], F32, tag="recip")
        nc.vector.reciprocal(r_s, tot)
        attnT2.append(aT)
        r_ss.append(r_s)

    for qb in range(NQB):
        outT = attn_pool.tile([P, DC, N5], CDT, tag="outT")
        for m in range(DC):
            ps = psum.tile([P, N5], F32, tag="mm")
            for c in range(KVHC):
                nc.tensor.matmul(
                    ps,
                    lhsT=vOther[:, c, m * P : (m + 1) * P],
                    rhs=attnT2[qb][:, c, :],
                    start=(c == 0),
                    stop=(c == KVHC - 1),
                )
            nc.vector.scalar_tensor_tensor(
                out=outT[:, m, :],
                in0=ps,
                scalar=1.0,
                in1=oacc[qb][:, m, :],
                op0=ALU.mult,
                op1=ALU.add,
            )

        # F[q, f] = O @ Wo ; out = F * (1/sums) + bo2
        for s in range(N5 // P):
            for nf in range(D // N5):
                ps = psum.tile([P, N5], F32, tag="mm")
                for m in range(DC):
                    nc.tensor.matmul(
                        ps,
                        lhsT=outT[:, m, s * P : (s + 1) * P],
                        rhs=wo_c[m][0][:, nf * N5 : (nf + 1) * N5],
                        start=(m == 0),
                        stop=(m == DC - 1),
                    )
                fin = evac.tile([P, N5], F32, tag="fin")
                nc.vector.scalar_tensor_tensor(
                    out=fin,
                    in0=ps,
                    scalar=r_ss[qb][:, s : s + 1],
                    in1=bo2_s[:, nf * N5 : (nf + 1) * N5],
                    op0=ALU.mult,
                    op1=ALU.add,
                )
                row0 = qb * N5 + s * P
                nc.sync.dma_start(
                    out=out[row0 : row0 + P, nf * N5 : (nf + 1) * N5], in_=fin
                )


def build_program():
    nc = bacc.Bacc(
        "TRN2", target_bir_lowering=False, debug=False, num_devices=NCORES
    )
    qT = nc.dram_tensor("qT", [D, QL], CDT, kind="ExternalInput").ap()
    kT = nc.dram_tensor("kT", [D, KVH], CDT, kind="ExternalInput").ap()
    vT = nc.dram_tensor("vT", [D, KVH], CDT, kind="ExternalInput").ap()
    wq = nc.dram_tensor("wq", [D, D], CDT, kind="ExternalInput").ap()
    wk = nc.dram_tensor("wk", [D, D], CDT, kind="ExternalInput").ap()
    wv = nc.dram_tensor("wv", [D, D], CDT, kind="ExternalInput").ap()
    wo = nc.dram_tensor("wo", [D, D], CDT, kind="ExternalInput").ap()
    bq = nc.dram_tensor("bq", [D], F32, kind="ExternalInput").ap()
    bk = nc.dram_tensor("bk", [D], F32, kind="ExternalInput").ap()
    bo2 = nc.dram_tensor("bo2", [D], F32, kind="ExternalInput").ap()
    mask = nc.dram_tensor("mask", [P, 2], F32, kind="ExternalInput").ap()
    out = nc.dram_tensor("out", [QL, D], F32, kind="ExternalOutput").ap()

    kg_half = nc.dram_tensor("kg_half", [P, DC, KVH], CDT).ap()
    kg_full = nc.dram_tensor("kg_full", [2, P, DC, KVH], CDT).ap()
    vg_half = nc.dram_tensor("vg_half", [P, KVHC, D], CDT).ap()
    vg_full = nc.dram_tensor("vg_full", [2, P, KVHC, D], CDT).ap()
    with tile.TileContext(nc) as tc:
        with ExitStack() as ctx:
            _build_tile(
                ctx,
                tc,
                (qT, kT, vT, wq, wk, wv, wo, bq, bk, bo2, mask, out),
                (kg_half, kg_full, vg_half, vg_full),
            )
    nc.compile()
    return nc


def prep_in_maps(query, key, value, Wq, bq, Wk, bk, Wv, bv, Wo, bo):
    """Host-side shard prep: slice, transpose to feature-major, cast."""
    query = np.asarray(query, np.float32)
    key = np.asarray(key, np.float32)
    value = np.asarray(value, np.float32)
    shared = {
        "wq": np.asarray(Wq, np.float32).astype(NP_CDT),
        "wk": np.asarray(Wk, np.float32).astype(NP_CDT),
        "wv": np.asarray(Wv, np.float32).astype(NP_CDT),
        "wo": np.asarray(Wo, np.float32).astype(NP_CDT),
        "bq": np.asarray(bq, np.float32),
        "bk": np.asarray(bk, np.float32),
        "bo2": (
            np.asarray(bv, np.float32) @ np.asarray(Wo, np.float32)
            + np.asarray(bo, np.float32)
        ),
    }
    # m[j] = 1.0 iff j is the OTHER rank in the pair: rank 0 -> [0,1], rank 1 -> [1,0]
    masks = [
        np.broadcast_to(np.array([[r, 1.0 - r]], np.float32), (P, 2)).copy()
        for r in (0.0, 1.0)
    ]
    in_maps = []
    for b in range(B):
        kTb = np.ascontiguousarray(key[b].T).astype(NP_CDT)
        vTb = np.ascontiguousarray(value[b].T).astype(NP_CDT)
        for h in range(2):
            qTb = np.ascontiguousarray(query[b, h * QL : (h + 1) * QL].T).astype(
                NP_CDT
            )
            in_maps.append(
                {
                    "qT": qTb,
                    "kT": kTb[:, h * KVH : (h + 1) * KVH],
                    "vT": vTb[:, h * KVH : (h + 1) * KVH],
                    "mask": masks[h],
                    **shared,
                }
            )
    return in_maps


_NC_CACHE = None


def _get_nc():
    global _NC_CACHE
    if _NC_CACHE is None:
        _NC_CACHE = build_program()
    return _NC_CACHE


def run(inputs, **run_kwargs):
    nc = _get_nc()
    in_maps = prep_in_maps(**inputs)
    res = run_bass_kernel_spmd(nc, in_maps, core_ids=list(range(NCORES)), **run_kwargs)
    out = np.empty((B, SQ, D), np.float32)
    for b in range(B):
        for h in range(2):
            out[b, h * QL : (h + 1) * QL] = res.results[2 * b + h]["out"]
    return out, res


def kernel(query, key, value, Wq, bq, Wk, bk, Wv, bv, Wo, bo):
    out, _ = run(
        dict(
            query=query, key=key, value=value, Wq=Wq, bq=bq, Wk=Wk, bk=bk,
            Wv=Wv, bv=bv, Wo=Wo, bo=bo,
        )
    )
    return out


if __name__ == "__main__":
    rng = np.random.default_rng(0)
    ins = {
        "query": rng.standard_normal((B, SQ, D), dtype=np.float32),
        "key": rng.standard_normal((B, SKV, D), dtype=np.float32),
        "value": rng.standard_normal((B, SKV, D), dtype=np.float32),
        "Wq": (rng.standard_normal((D, D), dtype=np.float32) * 0.02),
        "bq": np.zeros(D, np.float32),
        "Wk": (rng.standard_normal((D, D), dtype=np.float32) * 0.02),
        "bk": np.zeros(D, np.float32),
        "Wv": (rng.standard_normal((D, D), dtype=np.float32) * 0.02),
        "bv": np.zeros(D, np.float32),
        "Wo": (rng.standard_normal((D, D), dtype=np.float32) * 0.02),
        "bo": np.zeros(D, np.float32),
    }
    out = kernel(**ins)
    print("kernel ran, out shape", out.shape)



# revision 4
# speedup vs baseline: 1.2880x; 1.2880x over previous
"""Trainium2 Bass kernel for single-head cross-attention (v3: folded weights).

Reference computation (B=4, Sq=Skv=2048, D=1024, fp32):
    Q = query @ Wq + bq ; K = key @ Wk + bk ; V = value @ Wv + bv
    out = softmax(Q K^T / sqrt(D)) V @ Wo + bo

Since no nonlinearity separates the projections from the score/output
matmuls, the host folds the weights (a static, per-model transform):
    M = Wq @ Wk.T        scores = (query @ M) @ key^T  (K proj eliminated)
    N = Wv @ Wo          out    = (attn @ value) @ N   (V proj eliminated)
Bias terms fold exactly: the per-kv offset key @ (Wk @ bq) becomes the
exp() activation bias; per-q offsets cancel against the softmax
denominator (we divide by the sums at the very end, so they never need
computing); bv @ Wo + bo is the output bias.

This removes 25% of the device FLOPs and, because each core can simply
be HANDED the full raw key/value for its batch, the K/V AllGathers of
v2 disappear entirely. Sharding: 8 shards = (batch b) x (query half h);
core 2*b+h computes output rows [h*1024,(h+1)*1024) of batch b. All
matmul operands are bf16 (fp8 DoubleRow was measured at 1.9e-2 rel err
against the 2e-2 budget - too close).

M is pre-scaled by 32 on the host so Q' = query @ 32M has entries O(15)
(fp32 PSUM doesn't care, but it keeps the bf16 store well-conditioned);
the exp scale absorbs the 2^-10.

Dataflow per core (all contractions land on SBUF partitions):
    Q'^T[e,q]  = M32.T @ qT        (lhsT=m32,  rhs=qT)
    S^T[kv,q]  = key @ Q'^T        (lhsT=kT,   rhs=Q'^T)
    A^T        = exp(S^T/1024 + t2s)            (unnormalized)
    AX^T[dv,q] = value.T @ A^T     (lhsT=xv,   rhs=A^T)
    sums[q,1]  = A @ ones          (lhsT=A^T,  rhs=ones)
    F[q,f]     = AX @ N            (lhsT=AX^T, rhs=n2)
    out        = F * (1/sums) + (bv @ Wo + bo)
"""

import sys

if "/opt/trn_rl_repo" not in sys.path:
    sys.path.insert(0, "/opt/trn_rl_repo")

from contextlib import ExitStack

import ml_dtypes
import numpy as np

import concourse.bass as bass
import concourse.mybir as mybir
import concourse.tile as tile
from concourse import bacc
from concourse.bass_utils import run_bass_kernel_spmd

B, SQ, SKV, D = 4, 2048, 2048, 1024
NCORES = 8
QL = SQ // 2  # local query rows per core
P = 128
DC = D // P  # feature chunks (8)
KVC = SKV // P  # kv chunks (16)
N5 = 512
NQB = QL // N5  # query blocks (2)
F32 = mybir.dt.float32
CDT = mybir.dt.bfloat16
NP_CDT = ml_dtypes.bfloat16
MS = 32.0  # host pre-scale on M
SCALE = 1.0 / (32.0 * MS)  # exp scale: 1/sqrt(D) / MS

AF = mybir.ActivationFunctionType
ALU = mybir.AluOpType


def _build_tile(ctx: ExitStack, tc, aps):
    nc = tc.nc
    qT, kT, xv, m32, n2, t2s, bo2, out = aps

    big = ctx.enter_context(tc.tile_pool(name="big", bufs=1))
    attn_pool = ctx.enter_context(tc.tile_pool(name="attn", bufs=2))
    evac = ctx.enter_context(tc.tile_pool(name="evac", bufs=4))
    psum = ctx.enter_context(tc.tile_pool(name="psum", bufs=4, space="PSUM"))
    psum_s = ctx.enter_context(tc.tile_pool(name="psum_s", bufs=2, space="PSUM"))

    # ---- input DMAs, spread across rings so they stream in parallel --------
    # Critical path: the first Q' psum group consumes every d-chunk of m32,
    # so m32 rides two rings (evens/odds) and qT's first query block leads
    # the third; kT (scores, needed ~15us in) gets the sync ring to itself.
    m32_r = m32.rearrange("(c p) e -> p c e", p=P)
    qT_r = qT.rearrange("(c p) q -> p c q", p=P)
    kT_r = kT.rearrange("(c p) n -> p c n", p=P)
    xv_r = xv.rearrange("(c p) n -> p c n", p=P)
    n2_r = n2.rearrange("(c p) f -> p c f", p=P)

    m32_s = big.tile([P, DC, D], CDT, tag="m32")
    for dc in range(0, DC, 2):
        nc.scalar.dma_start(out=m32_s[:, dc, :], in_=m32_r[:, dc, :])
        nc.gpsimd.dma_start(out=m32_s[:, dc + 1, :], in_=m32_r[:, dc + 1, :])

    qT_s = big.tile([P, DC, QL], CDT, tag="qT")
    for qb in range(NQB):
        sl = slice(qb * N5, (qb + 1) * N5)
        nc.sync.dma_start(out=qT_s[:, :, sl], in_=qT_r[:, :, sl])

    kT_s = big.tile([P, DC, SKV], CDT, tag="kT")
    for j in range(4):
        sl = slice(j * (SKV // 4), (j + 1) * (SKV // 4))
        nc.sync.dma_start(out=kT_s[:, :, sl], in_=kT_r[:, :, sl])

    xv_s = big.tile([P, KVC, D], CDT, tag="xv")
    for j in range(4):
        sl = slice(j * (KVC // 4), (j + 1) * (KVC // 4))
        nc.scalar.dma_start(out=xv_s[:, sl, :], in_=xv_r[:, sl, :])

    t2s_s = big.tile([P, KVC], F32, tag="t2s")
    nc.gpsimd.dma_start(out=t2s_s, in_=t2s.rearrange("(c p) -> p c", p=P))
    bo2_s = big.tile([P, D], F32, tag="bo2")
    bo2_bcast = bass.AP(tensor=bo2.tensor, offset=bo2.offset, ap=[[0, P], bo2.ap[0]])
    nc.gpsimd.dma_start(out=bo2_s, in_=bo2_bcast)
    ones = big.tile([P, 1], CDT, tag="ones")
    nc.vector.memset(ones, 1.0)

    n2_s = big.tile([P, DC, D], CDT, tag="n2")
    for dc in range(0, DC, 2):
        nc.gpsimd.dma_start(out=n2_s[:, dc : dc + 2, :], in_=n2_r[:, dc : dc + 2, :])

    # ---- Q'^T = M32.T @ qT ---------------------------------------------------
    qp = big.tile([P, DC, QL], CDT, tag="qp")  # Q'^T: [d'%128, d'//128, q]

    def qprime(qb):
        for ec in range(DC):
            ps = psum.tile([P, N5], F32, tag="mm")
            for dc in range(DC):
                nc.tensor.matmul(
                    ps,
                    lhsT=m32_s[:, dc, ec * P : (ec + 1) * P],
                    rhs=qT_s[:, dc, qb * N5 : (qb + 1) * N5],
                    start=(dc == 0),
                    stop=(dc == DC - 1),
                )
            nc.scalar.activation(
                out=qp[:, ec, qb * N5 : (qb + 1) * N5],
                in_=ps,
                func=AF.Identity,
                scale=1.0,
            )

    # ---- scores + exp + sums + AX, one kv pass per 512-query block ----------
    def scores_exp(qb):
        attnT = attn_pool.tile([P, KVC, N5], CDT, tag="attnT")
        for c in range(KVC):
            ps = psum.tile([P, N5], F32, tag="mm")
            for dc in range(DC):
                nc.tensor.matmul(
                    ps,
                    lhsT=kT_s[:, dc, c * P : (c + 1) * P],
                    rhs=qp[:, dc, qb * N5 : (qb + 1) * N5],
                    start=(dc == 0),
                    stop=(dc == DC - 1),
                )
            nc.scalar.activation(
                out=attnT[:, c, :],
                in_=ps,
                func=AF.Exp,
                scale=SCALE,
                bias=t2s_s[:, c : c + 1],
            )
        return attnT

    def sums_recip(attnT):
        ps_sum = psum_s.tile([P, N5 // P], F32, tag="sums")
        for s in range(N5 // P):
            for c in range(KVC):
                nc.tensor.matmul(
                    ps_sum[:, s : s + 1],
                    lhsT=attnT[:, c, s * P : (s + 1) * P],
                    rhs=ones[:, :1],
                    start=(c == 0),
                    stop=(c == KVC - 1),
                )
        r_s = evac.tile([P, N5 // P], F32, tag="recip")
        nc.vector.reciprocal(r_s, ps_sum)
        return r_s

    def ax_block(attnT):
        axT = attn_pool.tile([P, DC, N5], CDT, tag="axT")  # AX^T: [dv%128, m, q]
        for m in range(DC):
            ps = psum.tile([P, N5], F32, tag="mm")
            for c in range(KVC):
                nc.tensor.matmul(
                    ps,
                    lhsT=xv_s[:, c, m * P : (m + 1) * P],
                    rhs=attnT[:, c, :],
                    start=(c == 0),
                    stop=(c == KVC - 1),
                )
            nc.vector.tensor_copy(out=axT[:, m, :], in_=ps)
        return axT

    def out_block(qb, axT, r_s):
        for s in range(N5 // P):
            for nf in range(D // N5):
                ps = psum.tile([P, N5], F32, tag="mm")
                for m in range(DC):
                    nc.tensor.matmul(
                        ps,
                        lhsT=axT[:, m, s * P : (s + 1) * P],
                        rhs=n2_s[:, m, nf * N5 : (nf + 1) * N5],
                        start=(m == 0),
                        stop=(m == DC - 1),
                    )
                fin = evac.tile([P, N5], F32, tag="fin")
                nc.vector.scalar_tensor_tensor(
                    out=fin,
                    in0=ps,
                    scalar=r_s[:, s : s + 1],
                    in1=bo2_s[:, nf * N5 : (nf + 1) * N5],
                    op0=ALU.mult,
                    op1=ALU.add,
                )
                row0 = qb * N5 + s * P
                nc.sync.dma_start(
                    out=out[row0 : row0 + P, nf * N5 : (nf + 1) * N5], in_=fin
                )

    qprime(0)
    qprime(1)
    a0 = scores_exp(0)
    r0 = sums_recip(a0)
    x0 = ax_block(a0)
    a1 = scores_exp(1)
    r1 = sums_recip(a1)
    x1 = ax_block(a1)
    out_block(0, x0, r0)
    out_block(1, x1, r1)


def build_program():
    nc = bacc.Bacc(
        "TRN2", target_bir_lowering=False, debug=False, num_devices=NCORES
    )
    qT = nc.dram_tensor("qT", [D, QL], CDT, kind="ExternalInput").ap()
    kT = nc.dram_tensor("kT", [D, SKV], CDT, kind="ExternalInput").ap()
    xv = nc.dram_tensor("xv", [SKV, D], CDT, kind="ExternalInput").ap()
    m32 = nc.dram_tensor("m32", [D, D], CDT, kind="ExternalInput").ap()
    n2 = nc.dram_tensor("n2", [D, D], CDT, kind="ExternalInput").ap()
    t2s = nc.dram_tensor("t2s", [SKV], F32, kind="ExternalInput").ap()
    bo2 = nc.dram_tensor("bo2", [D], F32, kind="ExternalInput").ap()
    out = nc.dram_tensor("out", [QL, D], F32, kind="ExternalOutput").ap()

    with tile.TileContext(nc) as tc:
        with ExitStack() as ctx:
            _build_tile(ctx, tc, (qT, kT, xv, m32, n2, t2s, bo2, out))
    nc.compile()
    return nc


def prep_in_maps(query, key, value, Wq, bq, Wk, bk, Wv, bv, Wo, bo):
    """Host-side shard prep: fold weights, slice, transpose, cast."""
    query = np.asarray(query, np.float32)
    key = np.asarray(key, np.float32)
    value = np.asarray(value, np.float32)
    Wq = np.asarray(Wq, np.float32)
    Wk = np.asarray(Wk, np.float32)
    Wv = np.asarray(Wv, np.float32)
    Wo = np.asarray(Wo, np.float32)
    bq = np.asarray(bq, np.float32)
    bv = np.asarray(bv, np.float32)
    bo = np.asarray(bo, np.float32)

    M32 = (Wq @ Wk.T) * MS
    N2 = Wv @ Wo
    ck = Wk @ bq  # per-kv score offset direction; zero when bq == 0
    shared = {
        "m32": M32.astype(NP_CDT),
        "n2": N2.astype(NP_CDT),
        "bo2": bv @ Wo + bo,
    }
    in_maps = []
    for b in range(B):
        kTb = np.ascontiguousarray(key[b].T).astype(NP_CDT)
        xvb = value[b].astype(NP_CDT)
        t2sb = (SCALE * (key[b] @ ck)).astype(np.float32)
        for h in range(2):
            qTb = np.ascontiguousarray(query[b, h * QL : (h + 1) * QL].T).astype(
                NP_CDT
            )
            in_maps.append({"qT": qTb, "kT": kTb, "xv": xvb, "t2s": t2sb, **shared})
    return in_maps


_NC_CACHE = None


def _get_nc():
    global _NC_CACHE
    if _NC_CACHE is None:
        _NC_CACHE = build_program()
    return _NC_CACHE


def run(inputs, **run_kwargs):
    nc = _get_nc()
    in_maps = prep_in_maps(**inputs)
    res = run_bass_kernel_spmd(nc, in_maps, core_ids=list(range(NCORES)), **run_kwargs)
    out = np.empty((B, SQ, D), np.float32)
    for b in range(B):
        for h in range(2):
            out[b, h * QL : (h + 1) * QL] = res.results[2 * b + h]["out"]
    return out, res


def kernel(query, key, value, Wq, bq, Wk, bk, Wv, bv, Wo, bo):
    out, _ = run(
        dict(
            query=query, key=key, value=value, Wq=Wq, bq=bq, Wk=Wk, bk=bk,
            Wv=Wv, bv=bv, Wo=Wo, bo=bo,
        )
    )
    return out


if __name__ == "__main__":
    rng = np.random.default_rng(0)
    ins = {
        "query": rng.standard_normal((B, SQ, D), dtype=np.float32),
        "key": rng.standard_normal((B, SKV, D), dtype=np.float32),
        "value": rng.standard_normal((B, SKV, D), dtype=np.float32),
        "Wq": (rng.standard_normal((D, D), dtype=np.float32) * 0.02),
        "bq": np.zeros(D, np.float32),
        "Wk": (rng.standard_normal((D, D), dtype=np.float32) * 0.02),
        "bk": np.zeros(D, np.float32),
        "Wv": (rng.standard_normal((D, D), dtype=np.float32) * 0.02),
        "bv": np.zeros(D, np.float32),
        "Wo": (rng.standard_normal((D, D), dtype=np.float32) * 0.02),
        "bo": np.zeros(D, np.float32),
    }
    out = kernel(**ins)
    print("kernel ran, out shape", out.shape)
